# revision 26
# baseline (speedup 1.0000x reference)
"""GatedDeltaNet attention kernel for 8 Trainium2 NeuronCores.

Problem: B=2, L=2048, D=1024, H=16 heads (Dh=64).
  q,k,v = x@Wq, x@Wk, x@Wv ; beta = sigmoid(x@Wb + bb)
  q,k l2-normalized per head; out[l] = sum_{t<=l} beta_t <qh_l,kh_t> vh_t
  y = out @ Wo

Sharding: 8 cores = 2 batches x 4 head-groups (4 heads each). Each core
computes its batch/heads slice end-to-end including a partial y (contraction
over its 256 Wo rows); host sums the 4 partials per batch.

Device algorithm (per core), bf16 matmul operands, f32 PSUM accumulation:
  P1: qT/kT = W^T projections into [d', l] layout (lhsT=W, rhs=xT), c-major
      so matmuls chase the x DMA chunks; per-head squared norms of q AND k
      land in one PSUM bank via [l,h]-layout selector matmuls, one
      Abs_reciprocal_sqrt produces all 1/|q|,1/|k| factors. v projection
      (beta logits fused as 4 extra columns) into [t, e]; 1/|k| and beta
      fold into v ("vtilde"). k in [t, d] layout for the state path comes
      from PE transposes of kT (not a second GEMM).
  P2: chunked DeltaNet: per chunk, score tiles ST[t,l] per (pair,head-half),
      causal diagonal handled by one mask-multiply on the PSUM->SBUF copy;
      out2 accumulated in [l, e] layout (64-wide free dim = half the PE
      cost of the [e, l] layout), inter-chunk state S applied the same way;
      1/|q| folds into the o2->attn copy; attnT recovered by PE transposes.
  P3: yT = Wo^T @ attnT per chunk, interleaved into the next chunk's P2 as
      PE filler; bf16 output halves the out-DMA and the final drain.
"""

import numpy as np

P = 128
L = 2048
D = 1024
H = 16
KS = D // P        # 8 contraction subtiles
NT = L // P        # 16 t-blocks
CH = 512
NCH = L // CH      # 4 l-chunks
DH = 64
HC = 4             # heads per core
NP = HC // 2       # head pairs per core
NCORES = 8
GROUPS = NCORES // 2  # head groups (4)
NV = HC * DH       # 256
NKN = NT - NT // NCH  # 12 t-blocks that enter the state

_CACHE = {}
DEBUG_DUMP = False


def _build_nc():
    import concourse.bass as bass  # noqa: F401
    import concourse.tile as tile
    import concourse.mybir as mybir
    from concourse import bacc
    from contextlib import ExitStack

    F32 = mybir.dt.float32
    F32R = mybir.dt.float32r
    BF16 = mybir.dt.bfloat16
    AF = mybir.ActivationFunctionType
    OP = mybir.AluOpType

    nc = bacc.Bacc(
        "TRN2", target_bir_lowering=False, debug=False, num_devices=NCORES
    )

    xT = nc.dram_tensor("xT", [KS, P, L], BF16, kind="ExternalInput")
    wq = nc.dram_tensor("wq", [KS, P, NP * P], BF16, kind="ExternalInput")
    wk = nc.dram_tensor("wk", [KS, P, NP * P], BF16, kind="ExternalInput")
    wv = nc.dram_tensor("wv", [KS, P, NV], BF16, kind="ExternalInput")
    wb = nc.dram_tensor("wb", [KS, P, HC], BF16, kind="ExternalInput")
    wo = nc.dram_tensor("wo", [NP, P, D], BF16, kind="ExternalInput")
    sel = nc.dram_tensor("sel", [P, 2], F32R, kind="ExternalInput")
    bbb = nc.dram_tensor("bbb", [P, HC], F32, kind="ExternalInput")
    masks = nc.dram_tensor("masks", [P, P], F32, kind="ExternalInput")
    eye = nc.dram_tensor("eye", [P, P], BF16, kind="ExternalInput")
    yT = nc.dram_tensor("yT", [D, L], BF16, kind="ExternalOutput")
    dbg = {}
    if DEBUG_DUMP:
        for nm, shp, dt_ in [
            ("d_qT0", [P, L], BF16), ("d_kT0", [P, L], BF16),
            ("d_vt", [P, NT * NV], BF16), ("d_kn", [P, NKN * NV], BF16),
            ("d_factor", [P, NT * HC], F32), ("d_rnkq", [P, NT * 8], F32),
            ("d_attnT0", [P, L], BF16),
        ]:
            dbg[nm] = nc.dram_tensor(nm, shp, dt_, kind="ExternalOutput")

    with tile.TileContext(nc) as tc:
        with ExitStack() as ctx:
            pconst = ctx.enter_context(tc.tile_pool(name="const", bufs=1))
            pmain = ctx.enter_context(tc.tile_pool(name="main", bufs=1))

            # ---- engine-load balancing for PSUM->SBUF copies / elementwise
            eng_load = {"dve": 0.0, "act": 0.0, "pool": 0.0}

            def cost_dve(fd, psum=True, all16=False):
                init = 120 if psum else 58
                return 1.0417 * (init / 2 + fd * (0.5 if all16 else 1.0)) + 70

            def cost_act(fd):
                return 0.8333 * (222 / 2 + fd) + 57

            def cost_pool(fd, mult=False):
                return 95 + 0.8333 * fd / (0.42 if mult else 0.6) + 61

            def pick(costs):
                # costs: list of (engine, cost); choose min accumulated
                best = min(costs, key=lambda ec: eng_load[ec[0]] + ec[1])
                eng_load[best[0]] += best[1]
                return best[0]

            def bal_copy(out_ap, in_ap, fd, all16=False, pool_ok=True):
                costs = [("dve", cost_dve(fd, all16=all16)),
                         ("act", cost_act(fd))]
                if pool_ok:
                    costs.append(("pool", cost_pool(fd)))
                e = pick(costs)
                if e == "dve":
                    nc.vector.tensor_copy(out_ap, in_ap)
                elif e == "act":
                    nc.scalar.activation(out_ap, in_ap, AF.Copy)
                else:
                    nc.gpsimd.tensor_copy(out_ap, in_ap)

            def bal_mult(out_ap, a_ap, b_ap, fd, pool_ok=True):
                costs = [("dve", cost_dve(fd))]
                if pool_ok:
                    costs.append(("pool", cost_pool(fd, mult=True)))
                e = pick(costs)
                if e == "dve":
                    nc.vector.tensor_tensor(out_ap, a_ap, b_ap, OP.mult)
                else:
                    nc.gpsimd.tensor_tensor(out_ap, a_ap, b_ap, OP.mult)

            # ---- constant tiles
            sel_sb = pconst.tile([P, 2], F32R, tag="sel", name="sel")
            bbb_sb = pconst.tile([P, HC], F32, tag="bbb", name="bbb")
            mask_sb = pconst.tile([P, P], F32, tag="mask", name="mask")
            eye_sb = pconst.tile([P, P], BF16, tag="eye", name="eye")
            wo_sb = pconst.tile([P, NP, D], BF16, tag="wo", name="wo")

            # ---- persistent SBUF tensors
            qT = [pmain.tile([P, L], BF16, tag=f"qT{p}", name=f"qT{p}")
                  for p in range(NP)]
            kT = [pmain.tile([P, L], BF16, tag=f"kT{p}", name=f"kT{p}")
                  for p in range(NP)]
            vt = pmain.tile([P, NT, NV], BF16, tag="vt", name="vt")
            kn = pmain.tile([P, NKN, NV], BF16, tag="kn", name="kn")
            factor = pmain.tile([P, NT, HC], F32, tag="factor", name="factor")
            # 1/|q| (cols 0:4 = pair*2+hh) and 1/|k| (cols 4:8), per t-block
            rnkq = pmain.tile([P, NT, 8], F32, tag="rnkq", name="rnkq")
            attnT = [pmain.tile([P, L], BF16, tag=f"attnT{p}", name=f"attnT{p}")
                     for p in range(NP)]
            s_sb = pmain.tile([P, NP, NV], BF16, tag="ssb", name="ssb")

            # weights + x
            wq_sb = pmain.tile([P, KS, NP * P], BF16, tag="wq", name="wq")
            wk_sb = pmain.tile([P, KS, NP * P], BF16, tag="wk", name="wk")
            wv_sb = pmain.tile([P, KS, NV], BF16, tag="wv", name="wv")
            wb_sb = pmain.tile([P, KS, HC], BF16, tag="wb", name="wb")
            x_sb = [pmain.tile([P, L], BF16, tag=f"x{ks}", name=f"x{ks}")
                    for ks in range(KS)]

            # ---- input DMAs, issue order == consumption order (SP queue).
            # HWDGE serializes ~625ns per trigger, so later x chunks use
            # coarser granularity to keep the trigger count low.
            nc.sync.dma_start(wq_sb[:, 0:KS // 2, :],
                              wq.ap()[0:KS // 2].rearrange("k p c -> p k c"))
            for ks in range(KS):
                nc.sync.dma_start(
                    x_sb[ks][:, 0:CH], xT.ap()[ks][:, 0:CH])
                if ks == 1:
                    nc.sync.dma_start(
                        wq_sb[:, KS // 2:, :],
                        wq.ap()[KS // 2:].rearrange("k p c -> p k c"))
            nc.sync.dma_start(sel_sb[:], sel.ap())
            nc.sync.dma_start(bbb_sb[:], bbb.ap())
            for ks in range(KS):
                nc.sync.dma_start(
                    x_sb[ks][:, CH:2 * CH], xT.ap()[ks][:, CH:2 * CH])
            nc.sync.dma_start(wk_sb[:], wk.ap().rearrange("k p c -> p k c"))
            for ks in range(KS):
                nc.sync.dma_start(
                    x_sb[ks][:, 2 * CH:L], xT.ap()[ks][:, 2 * CH:L])
            nc.sync.dma_start(wv_sb[:], wv.ap().rearrange("k p c -> p k c"))
            nc.sync.dma_start(wb_sb[:], wb.ap().rearrange("k p c -> p k c"))
            nc.sync.dma_start(mask_sb[:], masks.ap())
            nc.sync.dma_start(eye_sb[:], eye.ap())
            nc.sync.dma_start(
                wo_sb[:], wo.ap().rearrange("s p d -> p s d"))

            # ---- SBUF pools
            psq = ctx.enter_context(tc.tile_pool(name="sq", bufs=3))
            ptmp = ctx.enter_context(tc.tile_pool(name="tmp", bufs=4))
            pst = ctx.enter_context(tc.tile_pool(name="stbuf", bufs=8))
            pal = ctx.enter_context(tc.tile_pool(name="attnl", bufs=4))
            pyout = ctx.enter_context(tc.tile_pool(name="yout", bufs=6))

            # ---- PSUM pools (8 banks: ppA 3 + ppO 2 + ppS 1 + ppY 2;
            #      ppN's bank is only live during P1 before ppY is used)
            ppA = ctx.enter_context(
                tc.tile_pool(name="ppA", bufs=3, space="PSUM"))
            ppO = ctx.enter_context(
                tc.tile_pool(name="ppO", bufs=2, space="PSUM"))
            ppS = ctx.enter_context(
                tc.tile_pool(name="ppS", bufs=1, space="PSUM"))

            s_ps = ppS.tile([P, NP, NV], F32, tag="sps", name="sps")

            # ================= P1: q/k projections + norms =================
            nsel = [0]

            def issue_norm_matmuls(sq_ap, c, wi, pair):
                for tr in range(CH // P):
                    tb = c * (CH // P) + tr
                    col = wi * 4 + pair * 2
                    nc.tensor.matmul(
                        normbank[:, tb, col:col + 2],
                        sq_ap[:, tr * P:(tr + 1) * P],
                        sel_sb[:],
                        start=(nsel[0] == 0),
                        stop=(nsel[0] == 4 * NP * NCH - 1),
                        skip_group_check=True,
                    )
                    nsel[0] += 1

            with ExitStack() as pnctx:
                ppN = pnctx.enter_context(
                    tc.tile_pool(name="ppN", bufs=1, space="PSUM"))
                normbank = ppN.tile([P, NT, 8], F32, tag="nb", name="nb")
                betabank = ppN.tile([P, NT, HC], F32, tag="bb", name="bb")

                # phase order chases the x DMA arrival (c0,c1 of q first);
                # selector matmuls are deferred one group so they never
                # stall the in-order PE queue on a pending Square
                pending_sel = []
                for c, wi in ((0, 0), (1, 0), (0, 1), (1, 1),
                              (2, 0), (2, 1), (3, 0), (3, 1)):
                    cs = slice(c * CH, (c + 1) * CH)
                    w_sb, dstT = (wq_sb, qT) if wi == 0 else (wk_sb, kT)
                    ps = [ppA.tile([P, CH], F32, tag="mm", name="mm")
                          for _ in range(NP)]
                    for ks in range(KS):
                        for pair in range(NP):
                            nc.tensor.matmul(
                                ps[pair][:],
                                w_sb[:, ks, pair * P:(pair + 1) * P],
                                x_sb[ks][:, cs],
                                start=(ks == 0),
                                stop=(ks == KS - 1),
                            )
                    for fn in pending_sel:
                        fn()
                    pending_sel = []
                    for pair in range(NP):
                        bal_copy(dstT[pair][:, cs], ps[pair][:], CH)
                        sq = psq.tile([P, CH], F32R, tag="sq", name="sq")
                        # square on ACT or DVE, whichever is freer
                        cd, ca = cost_dve(CH), cost_act(CH)
                        if eng_load["dve"] + cd <= eng_load["act"] + ca:
                            eng_load["dve"] += cd
                            nc.vector.tensor_tensor(
                                sq[:], ps[pair][:], ps[pair][:], OP.mult)
                        else:
                            eng_load["act"] += ca
                            nc.scalar.activation(
                                sq[:], ps[pair][:], AF.Square)
                        pending_sel.append(
                            lambda sq=sq, c=c, wi=wi, pair=pair:
                            issue_norm_matmuls(sq, c, wi, pair))

                for fn in pending_sel:
                    fn()
                pending_sel = []
                # beta logits GEMM (batched: all 16 t-blocks into one bank)
                for tb in range(NT):
                    for ks in range(KS):
                        nc.tensor.matmul(
                            betabank[:, tb, :],
                            x_sb[ks][:, tb * P:(tb + 1) * P],
                            wb_sb[:, ks, :],
                            start=(ks == 0),
                            stop=(ks == KS - 1),
                            skip_group_check=True,
                        )

                # all 64 selector matmuls done -> one activation for all
                # norms; beta chain batched so ACT needs only two table
                # loads (abs_recip + sigmoid), both hidden under v-phase PE
                nc.scalar.activation(
                    rnkq[:].rearrange("p a b -> p (a b)"),
                    normbank[:].rearrange("p a b -> p (a b)"),
                    AF.Abs_reciprocal_sqrt)
                # two 1283ns act-table loads accompany abs_recip + sigmoid
                eng_load["act"] += cost_act(NT * 8) + 2 * 1283.0
                bl_all = pmain.tile([P, NT, HC], F32, tag="bl", name="bl")
                nc.vector.tensor_tensor(
                    bl_all[:],
                    betabank[:],
                    bbb_sb[:, None, :].to_broadcast((P, NT, HC)),
                    OP.add)
                eng_load["dve"] += cost_dve(NT * HC)
                bs_all = pmain.tile([P, NT, HC], F32, tag="bs", name="bs")
                nc.scalar.activation(
                    bs_all[:].rearrange("p a b -> p (a b)"),
                    bl_all[:].rearrange("p a b -> p (a b)"),
                    AF.Sigmoid)
                eng_load["act"] += cost_act(NT * HC)
                nc.vector.tensor_tensor(
                    factor[:], bs_all[:], rnkq[:, :, 4:8], OP.mult)
                eng_load["dve"] += cost_dve(NT * HC, psum=False)

            # ppY created after ppN closed: peak PSUM stays at 8 banks
            ppY = ctx.enter_context(
                tc.tile_pool(name="ppY", bufs=2, space="PSUM"))

            # ---------------- v projection / kn transpose items ------------
            def v_item(tb):
                def run():
                    psv = ppA.tile([P, NV], F32, tag="mm", name="mmv")
                    for ks in range(KS):
                        nc.tensor.matmul(
                            psv[:],
                            x_sb[ks][:, tb * P:(tb + 1) * P],
                            wv_sb[:, ks, :],
                            start=(ks == 0),
                            stop=(ks == KS - 1),
                        )
                    bal_mult(
                        vt[:, tb, :].rearrange("p (h e) -> p h e", e=DH),
                        psv[:].rearrange("p (h e) -> p h e", e=DH),
                        factor[:, tb, :, None].to_broadcast((P, HC, DH)),
                        NV, pool_ok=False)
                return run

            def kn_item(tb, pair):
                def run():
                    trp = ppA.tile([P, P], BF16, tag="mm", name="mmt")
                    nc.tensor.matmul(
                        trp[:],
                        kT[pair][:, tb * P:(tb + 1) * P],
                        eye_sb[:],
                        is_transpose=True,
                    )
                    bal_copy(kn[:, tb, pair * P:(pair + 1) * P], trp[:],
                             P, all16=True)
                return run

            # ---------------- P3 items (yT chunk output) -------------------
            # yo tiles hold 4 m-blocks; one DMA writes [512, 512] of yT
            # (HWDGE triggers are 625ns each, so merge aggressively)
            yo_half = {}

            def p3_item(c, m):
                def run():
                    py = ppY.tile([P, CH], F32, tag="py", name="py")
                    for pair in range(NP):
                        nc.tensor.matmul(
                            py[:],
                            wo_sb[:, pair, m * P:(m + 1) * P],
                            attnT[pair][:, c * CH:(c + 1) * CH],
                            start=(pair == 0),
                            stop=(pair == NP - 1),
                        )
                    half, mi = divmod(m, 4)
                    if mi == 0:
                        yo_half[(c, half)] = pyout.tile(
                            [P, 4, CH], BF16, tag="yo", name="yo")
                    yo = yo_half[(c, half)]
                    bal_copy(yo[:, mi, :], py[:], CH)
                    if mi == 3:
                        nc.sync.dma_start(
                            yT.ap()[half * 4 * P:(half + 1) * 4 * P,
                                    c * CH:(c + 1) * CH]
                            .rearrange("(m p) c -> p m c", p=P),
                            yo[:],
                        )
                return run

            # eager: v+kn for chunk 0 (needed by P2 c0 / fold at c1)
            for tb in range(4):
                v_item(tb)()
            for tb in range(4):
                for pair in range(NP):
                    kn_item(tb, pair)()

            # filler queues per P2 chunk; popped evenly across the chunk's
            # fill points so late T-iterations don't starve
            fillers = {c: [] for c in range(NCH)}
            fillpts = {c: 16 for c in range(NCH)}
            for tb in range(4, 8):
                fillers[0].append(v_item(tb))
            for tb in range(4, 8):
                for pair in range(NP):
                    fillers[0].append(kn_item(tb, pair))
            for tb in range(8, 12):
                fillers[1].append(v_item(tb))
            for tb in range(8, 12):
                for pair in range(NP):
                    fillers[1].append(kn_item(tb, pair))
            for tb in range(12, 16):
                fillers[2].append(v_item(tb))
            for c in range(1, NCH):
                for m in range(D // P):
                    fillers[c].append(p3_item(c - 1, m))

            def pop_fill(c, pts=1):
                # consume a fair share of the remaining fillers
                n = -(-len(fillers[c]) // max(fillpts[c], 1)) * pts
                fillpts[c] = max(fillpts[c] - pts, 0)
                for _ in range(n):
                    if fillers[c]:
                        fillers[c].pop(0)()

            # ================= P2: chunked DeltaNet =================
            for c in range(NCH):
                o2l = {}
                als = {}
                for pair in range(NP):
                    o2l[pair] = ppO.tile([P, NCH, P], F32, tag="o2l",
                                         name=f"o2l{pair}")
                    als[pair] = pal.tile([P, NCH, P], BF16, tag="al",
                                         name="al")
                if c > 0:
                    # fold chunk c-1 into the state, snapshot to bf16
                    for pair in range(NP):
                        for tsub in range(4):
                            tb = (c - 1) * 4 + tsub
                            nc.tensor.matmul(
                                s_ps[:, pair, :],
                                kn[:, tb, pair * P:(pair + 1) * P],
                                vt[:, tb, :],
                                start=(c == 1 and tsub == 0),
                                stop=(c == NCH - 1 and tsub == 3),
                                skip_group_check=True,
                            )
                        bal_copy(s_sb[:, pair, :], s_ps[:, pair, :], NV,
                                 pool_ok=False)
                    pop_fill(c)
                    # inter-chunk: o2l[lb, hh*64:..] = qT^T @ S  (free dim 64)
                    for pair in range(NP):
                        for hh in range(2):
                            h = 2 * pair + hh
                            for lb in range(NCH):
                                nc.tensor.matmul(
                                    o2l[pair][:, lb, hh * DH:(hh + 1) * DH],
                                    qT[pair][
                                        64 * hh:64 * (hh + 1),
                                        c * CH + lb * P:c * CH + (lb + 1) * P,
                                    ],
                                    s_sb[64 * hh:64 * (hh + 1), pair,
                                         h * DH:(h + 1) * DH],
                                    start=True, stop=False,
                                    skip_group_check=True,
                                )
                for T in range(4 * c, 4 * c + 4):
                    j = T - 4 * c
                    lo = P * j
                    # score tiles ST[t, l] for both pairs/hh
                    stps = {}
                    for pair in range(NP):
                        for hh in range(2):
                            sp = ppA.tile([P, CH], F32, tag="mm", name="st")
                            nc.tensor.matmul(
                                sp[:, lo:CH],
                                kT[pair][64 * hh:64 * (hh + 1),
                                         T * P:(T + 1) * P],
                                qT[pair][64 * hh:64 * (hh + 1),
                                         c * CH + lo:(c + 1) * CH],
                                start=True, stop=True,
                            )
                            stps[(pair, hh)] = sp
                    pop_fill(c)
                    st_sb = {}
                    for pair in range(NP):
                        for hh in range(2):
                            sb = pst.tile([P, CH], BF16, tag="st_sb",
                                          name="st_sb")
                            # triangular block at the causal frontier
                            bal_mult(sb[:, lo:lo + P],
                                     stps[(pair, hh)][:, lo:lo + P],
                                     mask_sb[:], P, pool_ok=False)
                            if lo + P < CH:
                                bal_copy(sb[:, lo + P:CH],
                                         stps[(pair, hh)][:, lo + P:CH],
                                         CH - lo - P, pool_ok=False)
                            st_sb[(pair, hh)] = sb
                    pop_fill(c)
                    # out2 in [l, e]: o2l[lb] += ST[:, lb]^T @ vtilde
                    for pair in range(NP):
                        for hh in range(2):
                            h = 2 * pair + hh
                            for lb in range(j, NCH):
                                nc.tensor.matmul(
                                    o2l[pair][:, lb, hh * DH:(hh + 1) * DH],
                                    st_sb[(pair, hh)][:, lb * P:(lb + 1) * P],
                                    vt[:, T, h * DH:(h + 1) * DH],
                                    start=(c == 0 and j == 0),
                                    stop=(j == lb),
                                    skip_group_check=True,
                                )
                    pop_fill(c)
                    # o2l region lb=j just received its last write: fold in
                    # 1/|q| and transpose to attnT now, pipelined across Ts
                    lb = j
                    for pair in range(NP):
                        nc.vector.tensor_tensor(
                            als[pair][:, lb, :]
                            .rearrange("p (b e) -> p b e", e=DH),
                            o2l[pair][:, lb, :]
                            .rearrange("p (b e) -> p b e", e=DH),
                            rnkq[:, 4 * c + lb,
                                 2 * pair:2 * pair + 2, None]
                            .to_broadcast((P, 2, DH)),
                            OP.mult)
                        eng_load["dve"] += cost_dve(P)
                    for pair in range(NP):
                        trp = ppA.tile([P, P], BF16, tag="mm", name="altr")
                        nc.tensor.matmul(
                            trp[:], als[pair][:, lb, :], eye_sb[:],
                            is_transpose=True,
                        )
                        bal_copy(
                            attnT[pair][:, c * CH + lb * P:
                                        c * CH + (lb + 1) * P],
                            trp[:], P, all16=True)
                pop_fill(c, 2)

            # last chunk's P3: 2-m-block DMA granularity and no Pool
            # copies, to keep the final drain short
            c = NCH - 1
            yo2 = None
            for m in range(D // P):
                py = ppY.tile([P, CH], F32, tag="py", name="py")
                for pair in range(NP):
                    nc.tensor.matmul(
                        py[:],
                        wo_sb[:, pair, m * P:(m + 1) * P],
                        attnT[pair][:, c * CH:(c + 1) * CH],
                        start=(pair == 0),
                        stop=(pair == NP - 1),
                    )
                if m % 2 == 0:
                    yo2 = pyout.tile([P, 2, CH], BF16, tag="yo2", name="yo2")
                bal_copy(yo2[:, m % 2, :], py[:], CH, pool_ok=False)
                if m % 2 == 1:
                    nc.sync.dma_start(
                        yT.ap()[(m - 1) * P:(m + 1) * P,
                                c * CH:(c + 1) * CH]
                        .rearrange("(m p) c -> p m c", p=P),
                        yo2[:],
                    )

            if DEBUG_DUMP:
                nc.sync.dma_start(dbg["d_qT0"].ap(), qT[0][:])
                nc.sync.dma_start(dbg["d_kT0"].ap(), kT[0][:])
                nc.sync.dma_start(
                    dbg["d_vt"].ap(), vt[:].rearrange("p a b -> p (a b)"))
                nc.sync.dma_start(
                    dbg["d_kn"].ap(), kn[:].rearrange("p a b -> p (a b)"))
                nc.sync.dma_start(
                    dbg["d_factor"].ap(),
                    factor[:].rearrange("p a b -> p (a b)"))
                nc.sync.dma_start(
                    dbg["d_rnkq"].ap(),
                    rnkq[:].rearrange("p a b -> p (a b)"))
                nc.sync.dma_start(dbg["d_attnT0"].ap(), attnT[0][:])

    nc.compile()
    return nc


def get_nc():
    if "nc" not in _CACHE:
        _CACHE["nc"] = _build_nc()
    return _CACHE["nc"]


def make_core_inputs(x, Wq, Wk, Wv, Wo, Wb, bb):
    """Build the 8 per-core input maps from full inputs."""
    import ml_dtypes
    BF = ml_dtypes.bfloat16

    x = np.asarray(x, dtype=np.float32)
    Wq = np.asarray(Wq, dtype=np.float32)
    Wk = np.asarray(Wk, dtype=np.float32)
    Wv = np.asarray(Wv, dtype=np.float32)
    Wo = np.asarray(Wo, dtype=np.float32)
    Wb = np.asarray(Wb, dtype=np.float32)
    bb = np.asarray(bb, dtype=np.float32)

    selm = np.zeros((P, 2), dtype=np.float32)
    selm[:64, 0] = 1.0
    selm[64:, 1] = 1.0
    maskm = (np.arange(P)[:, None] <= np.arange(P)[None, :]).astype(np.float32)
    eyem = np.eye(P, dtype=BF)

    in_maps = []
    for core in range(NCORES):
        b, g = divmod(core, GROUPS)
        hs = slice(NV * g, NV * (g + 1))
        bs = slice(HC * g, HC * (g + 1))
        xTc = np.ascontiguousarray(x[b].T).reshape(KS, P, L).astype(BF)
        wqc = np.ascontiguousarray(Wq[:, hs]).reshape(KS, P, NP * P).astype(BF)
        wkc = np.ascontiguousarray(Wk[:, hs]).reshape(KS, P, NP * P).astype(BF)
        wvc = np.ascontiguousarray(Wv[:, hs]).reshape(KS, P, NV).astype(BF)
        wbc = np.ascontiguousarray(Wb[:, bs]).reshape(KS, P, HC).astype(BF)
        woc = np.ascontiguousarray(Wo[hs, :]).reshape(NP, P, D).astype(BF)
        bbbc = np.ascontiguousarray(np.tile(bb[bs][None, :], (P, 1)))
        in_maps.append(
            {
                "xT": xTc,
                "wq": wqc,
                "wk": wkc,
                "wv": wvc,
                "wb": wbc,
                "wo": woc,
                "sel": selm,
                "bbb": bbbc,
                "masks": maskm,
                "eye": eyem,
            }
        )
    return in_maps


def kernel(x, Wq, Wk, Wv, Wo, Wb, bb):
    from concourse.bass_utils import run_bass_kernel_spmd

    nc = get_nc()
    in_maps = make_core_inputs(x, Wq, Wk, Wv, Wo, Wb, bb)
    try:
        res = run_bass_kernel_spmd(nc, in_maps, core_ids=list(range(NCORES)))
    except Exception:
        # transient NRT wedges (e.g. NRT_EXEC_UNIT_UNRECOVERABLE) clear on
        # a fresh attempt; retry once before giving up
        res = run_bass_kernel_spmd(nc, in_maps, core_ids=list(range(NCORES)))
    B = 2
    y = np.zeros((B, L, D), dtype=np.float32)
    for core in range(NCORES):
        b = core // GROUPS
        y[b] += np.asarray(res.results[core]["yT"]).astype(np.float32).T
    return y


if __name__ == "__main__":
    rng = np.random.default_rng(0)
    ins = {
        "x": rng.standard_normal((2, L, D)).astype(np.float32),
        "Wq": (0.02 * rng.standard_normal((D, D))).astype(np.float32),
        "Wk": (0.02 * rng.standard_normal((D, D))).astype(np.float32),
        "Wv": (0.02 * rng.standard_normal((D, D))).astype(np.float32),
        "Wo": (0.02 * rng.standard_normal((D, D))).astype(np.float32),
        "Wb": (0.02 * rng.standard_normal((D, H))).astype(np.float32),
        "bb": np.zeros(H, dtype=np.float32),
    }
    out = kernel(**ins)
    print("kernel ran, out shape", out.shape, "mean abs", np.abs(out).mean())
